# revision 7
# baseline (speedup 1.0000x reference)
"""Causal self-attention (B=4, T=2048, C=1024, H=16) on 8 trn2 NeuronCores.

Sharding: core c -> (batch b = c//2, head-group hg = c%2). Each core computes
q/k/v projections for its 8 heads only (no duplicated K/V work), runs full
causal attention for those heads over all T=2048 queries, and produces a
PARTIAL output projection (contracting its 512 of 1024 y-dims against the
matching Wp rows). The host sums the two partials per batch and adds the
output bias. All cores run an identical SPMD program.

Device pipeline (bf16 matmuls, fp32 PSUM):
  - Warm-up matmuls run during the initial input DMA so the PE clock gate
    (HAM) is released before real work arrives; inputs stream on two DMA
    queues (sync + gpsimd).
  - qT/kT projections in transposed layout [d, t]; v in natural layout
    [t, d] + ones column per head (AV matmul then also yields softmax Z).
  - Attention per head-pair as one flat software-pipelined stream over
    (J, kt) steps: S^T = K Q^T row-packed (tile_position), exp on the scalar
    engine straight out of PSUM for 2/3 of key tiles and as a one-instruction
    Schraudolph bit-trick exp on the vector engine (f32 -> int16 bits
    reinterpreted as bf16) for the remaining 1/3, causal diagonal via
    multiplicative bf16 masks on the gpsimd engine, AV accumulated over key
    tiles in PSUM with 128-granular causal trimming. The AV for step i is
    emitted after step i+1's S/exp so the tensor engine never waits on exp;
    the pipeline runs across J-block boundaries.
  - Projections for later head pairs and deferred softmax normalization are
    interleaved into earlier attention loops; the output projection is
    interleaved per-J into the LAST pair's attention as soon as that J's
    rows are normalized. Partial [2048, 1024] f32 output DMAs alternate
    between two queues.
"""

import numpy as np
import ml_dtypes
from contextlib import ExitStack

import concourse.bass as bass
import concourse.bacc as bacc
import concourse.mybir as mybir
import concourse.tile as tile
from concourse import bass_utils

B, T, C, H = 4, 2048, 1024, 16
HD = C // H            # 64
NCORES = 8
HPC = H // 2           # 8 heads per core
NCH = C // 128         # 8 contraction chunks of x
SCALE = 1.0 / float(np.sqrt(HD))
EXP_A = float(128.0 / np.log(2.0)) * SCALE   # Schraudolph scale (into bf16 bits)
EXP_B = float(16256.0 - 128.0 * 0.0575)      # Schraudolph offset (mean-zero)

bf16 = mybir.dt.bfloat16
f32 = mybir.dt.float32
i16 = mybir.dt.int16
AF = mybir.ActivationFunctionType
ALU = mybir.AluOpType

_compiled = {}
last_result = None  # BassKernelResults of the most recent run (for test harness)


def _build():
    nc = bacc.Bacc("TRN2", target_bir_lowering=False, debug=False,
                   num_devices=NCORES)

    xT_d = nc.dram_tensor("xT", [C, T], bf16, kind="ExternalInput")
    wqT_d = nc.dram_tensor("wqT", [C, 512], bf16, kind="ExternalInput")
    wkT_d = nc.dram_tensor("wkT", [C, 512], bf16, kind="ExternalInput")
    wvT_d = nc.dram_tensor("wvT", [C, 512], bf16, kind="ExternalInput")
    wpT_d = nc.dram_tensor("wpT", [512, C], bf16, kind="ExternalInput")
    bq_d = nc.dram_tensor("bq2", [128, 4], f32, kind="ExternalInput")
    bk_d = nc.dram_tensor("bk2", [128, 4], f32, kind="ExternalInput")
    bv_d = nc.dram_tensor("bv2", [1, 512], bf16, kind="ExternalInput")
    mask_d = nc.dram_tensor("mask", [512, 512], bf16, kind="ExternalInput")
    out_d = nc.dram_tensor("out", [T, C], f32, kind="ExternalOutput")

    xT_v = xT_d.ap().rearrange("(a p) t -> a p t", p=128)
    wq_v = wqT_d.ap().rearrange("(a p) o -> a p o", p=128)
    wk_v = wkT_d.ap().rearrange("(a p) o -> a p o", p=128)
    wv_v = wvT_d.ap().rearrange("(a p) o -> a p o", p=128)
    wp_v = wpT_d.ap().rearrange("(a p) o -> a p o", p=128)
    mask_v = mask_d.ap().rearrange("(a p) i -> a p i", p=128)

    with tile.TileContext(nc) as tc, ExitStack() as ctx:
        persist = ctx.enter_context(tc.tile_pool(name="persist", bufs=1))
        pp = ctx.enter_context(tc.tile_pool(name="pp", bufs=2, space="PSUM"))
        spool = ctx.enter_context(tc.tile_pool(name="spool", bufs=2,
                                               space="PSUM"))
        opool = ctx.enter_context(tc.tile_pool(name="opool", bufs=1,
                                               space="PSUM"))
        ppool = ctx.enter_context(tc.tile_pool(name="ppool", bufs=3))
        outp = ctx.enter_context(tc.tile_pool(name="outp", bufs=3))

        xT_sb = persist.tile([128, NCH, T], bf16)
        qT_sb = persist.tile([128, 4, T], bf16)
        kT_sb = persist.tile([128, 4, T], bf16)
        v_sb = persist.tile([128, 16, HPC, HD + 1], bf16)
        yT_sb = persist.tile([128, 4, T], bf16)
        wq_sb = persist.tile([128, NCH, 512], bf16)
        wk_sb = persist.tile([128, NCH, 512], bf16)
        wv_sb = persist.tile([128, NCH, 512], bf16)
        wp_sb = persist.tile([128, 4, C], bf16)
        bq_sb = persist.tile([128, 4], f32)
        bk_sb = persist.tile([128, 4], f32)
        bv_sb = persist.tile([1, 512], bf16)
        mask_sb = persist.tile([128, 4, 512], bf16)
        zst = persist.tile([128, HPC, 512], bf16)   # Z at row 32J, plane h
        ones_m = persist.tile([1, 128], bf16)    # v-bias broadcast matmul
        ones_r = persist.tile([128, 64], bf16)   # 1/Z broadcast matmul
        warm_w = persist.tile([128, 512], bf16)  # HAM warm-up fodder

        nc.vector.memset(ones_m[:], 1.0)
        nc.vector.memset(ones_r[:], 1.0)
        nc.vector.memset(warm_w[:], 0.125)
        nc.vector.memset(v_sb[:, :, :, HD:HD + 1], 1.0)  # aug ones column
        nc.vector.memset(zst[:], 1.0)

        # input DMAs on two queues: sync carries xT, gpsimd the weights
        for c in range(NCH):
            nc.sync.dma_start(xT_sb[:, c, :], xT_v[c])
            nc.gpsimd.dma_start(wq_sb[:, c, :], wq_v[c])
            nc.gpsimd.dma_start(wk_sb[:, c, :], wk_v[c])
        nc.sync.dma_start(bq_sb[:], bq_d.ap())
        nc.sync.dma_start(bk_sb[:], bk_d.ap())
        nc.sync.dma_start(bv_sb[:], bv_d.ap())
        for c in range(NCH):
            nc.gpsimd.dma_start(wv_sb[:, c, :], wv_v[c])
        for m in range(4):
            nc.gpsimd.dma_start(mask_sb[:, m, :], mask_v[m])
        for c in range(4):
            nc.gpsimd.dma_start(wp_sb[:, c, :], wp_v[c])

        # PE warm-up during the input DMA window (junk matmuls)
        for _ in range(16):
            ps = pp.tile([128, 512], f32, tag="pp")
            nc.tensor.matmul(ps[:], warm_w[:, 0:128], warm_w[:],
                             start=True, stop=True)

        # ---------------- emission helpers ----------------
        def qk_unit(hp, tn, t4):
            w_sb, b_sb, dst = (wq_sb, bq_sb, qT_sb) if tn == 0 else \
                              (wk_sb, bk_sb, kT_sb)
            ps = pp.tile([128, 512], f32, tag="pp")
            for c in range(NCH):
                nc.tensor.matmul(
                    ps[:], w_sb[:, c, 128 * hp:128 * hp + 128],
                    xT_sb[:, c, 512 * t4:512 * t4 + 512],
                    start=(c == 0), stop=(c == NCH - 1))
            nc.vector.tensor_scalar_add(
                dst[:, hp, 512 * t4:512 * t4 + 512], ps[:], b_sb[:, hp:hp + 1])

        def v_unit(r):
            ps = pp.tile([128, 512], f32, tag="pp")
            for c in range(NCH):
                nc.tensor.matmul(
                    ps[:], xT_sb[:, c, 128 * r:128 * r + 128], wv_sb[:, c, :],
                    start=(c == 0), stop=False)
            nc.tensor.matmul(ps[:], ones_m[:], bv_sb[:],
                             start=False, stop=True)
            nc.vector.tensor_copy(
                v_sb[:, r, :, 0:HD],
                ps[:].rearrange("p (h e) -> p h e", e=HD))

        def norm_unit(h, J):
            b = 32 * J
            qs = slice(512 * J, 512 * J + 512)
            bp1 = pp.tile([64, 512], f32, tag="pp")
            nc.tensor.matmul(bp1[:], ones_r[b:b + 1, :], zst[b:b + 1, h, :],
                             tile_position=(b, 0))
            nc.vector.reciprocal_approx_fast(bp1[:], bp1[:])
            pr = 64 * (h % 2)
            nc.vector.tensor_mul(yT_sb[pr:pr + 64, h // 2, qs],
                                 yT_sb[pr:pr + 64, h // 2, qs], bp1[:])

        def p_unit(qt, co):
            ps = pp.tile([128, 512], f32, tag="pp")
            for c2 in range(4):
                nc.tensor.matmul(
                    ps[:], yT_sb[:, c2, 128 * qt:128 * qt + 128],
                    wp_sb[:, c2, 512 * co:512 * co + 512],
                    start=(c2 == 0), stop=(c2 == 3))
            ot = outp.tile([128, 512], f32, tag="ot")
            if co == 0:
                nc.vector.tensor_copy(ot[:], ps[:])
            else:
                nc.scalar.activation(ot[:], ps[:], AF.Copy)
            eng = nc.sync if (qt + co) % 2 == 0 else nc.gpsimd
            eng.dma_start(
                out_d.ap()[128 * qt:128 * qt + 128,
                           512 * co:512 * co + 512], ot[:])

        def attention_pair(hp, tasks, jhook=None):
            steps = [(J, kt) for J in range(4) for kt in range(4 * (J + 1))]
            n = len(steps)
            state = {"emitted": 0, "cur": None, "pend": None}

            def drain_to(k):
                while state["emitted"] < min(k, len(tasks)):
                    tasks[state["emitted"]]()
                    state["emitted"] += 1

            def flush():
                J, kt, p2, i0 = state["pend"]
                state["pend"] = None
                E = 4 * (J + 1)
                if kt == 0:
                    oA = opool.tile([HD + 1, 512], f32, tag="oA", name="oA")
                    oB = opool.tile([HD + 1, 512], f32, tag="oB", name="oB")
                    state["cur"] = (oA, oB)
                oA, oB = state["cur"]
                last = (kt == E - 1)
                nc.tensor.matmul(oA[:, i0:512], v_sb[:, kt, 2 * hp, :],
                                 p2[:, i0:512], start=(kt == 0), stop=last)
                nc.tensor.matmul(oB[:, i0:512], v_sb[:, kt, 2 * hp + 1, :],
                                 p2[:, 512 + i0:1024],
                                 start=(kt == 0), stop=last)
                if last:
                    qs = slice(512 * J, 512 * J + 512)
                    nc.vector.tensor_copy(yT_sb[0:64, hp, qs], oA[0:HD, :])
                    nc.vector.tensor_copy(yT_sb[64:128, hp, qs], oB[0:HD, :])
                    nc.vector.tensor_copy(zst[32 * J:32 * J + 1, 2 * hp, :],
                                          oA[HD:HD + 1, :])
                    nc.vector.tensor_copy(
                        zst[32 * J:32 * J + 1, 2 * hp + 1, :],
                        oB[HD:HD + 1, :])
                    if jhook is not None:
                        jhook(J, drain_to)

            for idx, (J, kt) in enumerate(steps):
                ks = slice(128 * kt, 128 * kt + 128)
                i0 = 128 * (kt - 4 * J) if kt >= 4 * J else 0
                s2 = spool.tile([128, 1024], f32, tag="s2")
                nc.tensor.matmul(s2[:, i0:512], kT_sb[0:64, hp, ks],
                                 qT_sb[0:64, hp, 512 * J + i0:512 * J + 512],
                                 tile_position=(0, 0))
                nc.tensor.matmul(s2[:, 512 + i0:1024], kT_sb[64:128, hp, ks],
                                 qT_sb[64:128, hp,
                                       512 * J + i0:512 * J + 512],
                                 tile_position=(64, 0))
                p2 = ppool.tile([128, 1024], bf16, tag="p2")
                s2v = s2[:].rearrange("p (h q) -> p h q", q=512)
                p2v = p2[:].rearrange("p (h q) -> p h q", q=512)
                if kt % 3 == 1:   # Schraudolph exp on the vector engine
                    nc.vector.tensor_scalar(
                        p2v[:, :, i0:512].bitcast(i16), s2v[:, :, i0:512],
                        EXP_A, EXP_B, ALU.mult, ALU.add)
                else:             # spline exp on the scalar engine
                    nc.scalar.activation(p2v[:, :, i0:512], s2v[:, :, i0:512],
                                         AF.Exp, scale=SCALE)
                if kt >= 4 * J:  # diagonal block: causal mask (gpsimd)
                    m = kt - 4 * J
                    nc.gpsimd.tensor_mul(p2[:, i0:512], p2[:, i0:512],
                                         mask_sb[:, m, i0:512])
                    nc.gpsimd.tensor_mul(p2[:, 512 + i0:1024],
                                         p2[:, 512 + i0:1024],
                                         mask_sb[:, m, i0:512])
                drain_to(len(tasks) * (idx + 1) // n)
                if state["pend"] is not None:
                    flush()
                state["pend"] = (J, kt, p2, i0)
            flush()
            drain_to(len(tasks))
            return drain_to

        # ---------------- schedule ----------------
        # prologue: projections for pair 0, first half of v
        for tn in range(2):
            for t4 in range(4):
                qk_unit(0, tn, t4)
        for r in range(8):
            v_unit(r)

        def mk_v(r):
            return lambda: v_unit(r)

        def mk_qk(hp, tn, t4):
            return lambda: qk_unit(hp, tn, t4)

        def mk_norm(h, J):
            return lambda: norm_unit(h, J)

        # pair 0: rest of v + projections for pair 1
        attention_pair(0, [mk_v(r) for r in range(8, 16)] +
                       [mk_qk(1, tn, t4) for tn in range(2)
                        for t4 in range(4)])
        # pair 1: projections for pair 2, then pair-0 normalization
        attention_pair(1, [mk_qk(2, tn, t4) for tn in range(2)
                           for t4 in range(4)] +
                       [mk_norm(h, J) for h in (0, 1) for J in range(4)])
        # pair 2: projections for pair 3, then pair-1 normalization
        attention_pair(2, [mk_qk(3, tn, t4) for tn in range(2)
                           for t4 in range(4)] +
                       [mk_norm(h, J) for h in (2, 3) for J in range(4)])

        # pair 3: pair-2 normalization tasks (J-major so the jhook can
        # force-drain them per J), plus per-J appended work: pair-3
        # normalization and the output projection of that J's token rows
        p3_tasks = [mk_norm(h, J) for J in range(4) for h in (4, 5)]

        def mk_p(qt, co):
            return lambda: p_unit(qt, co)

        def jhook3(J, drain_to):
            drain_to(2 * J + 2)          # norm(4, J), norm(5, J) done
            p3_tasks.append(mk_norm(6, J))
            p3_tasks.append(mk_norm(7, J))
            for qt in range(4 * J, 4 * J + 4):
                for co in range(2):
                    p3_tasks.append(mk_p(qt, co))

        attention_pair(3, p3_tasks, jhook=jhook3)

    nc.compile()
    return nc


def prep_in_maps(x, Wq, bq, Wk, bk, Wv, bv, Wp, bp):
    x = np.asarray(x, dtype=np.float32)
    Wq = np.asarray(Wq, dtype=np.float32)
    Wk = np.asarray(Wk, dtype=np.float32)
    Wv = np.asarray(Wv, dtype=np.float32)
    Wp = np.asarray(Wp, dtype=np.float32)
    bq = np.asarray(bq, dtype=np.float32)
    bk = np.asarray(bk, dtype=np.float32)
    bv = np.asarray(bv, dtype=np.float32)

    bf = ml_dtypes.bfloat16
    WqT, WkT, WvT, WpT = Wq.T, Wk.T, Wv.T, Wp.T

    kk = np.arange(128)[:, None]
    qq = np.arange(512)[None, :]
    mask = np.ascontiguousarray(np.concatenate(
        [(128 * m + kk <= qq) for m in range(4)], axis=0).astype(bf))

    xTs = [np.ascontiguousarray(x[b].T).astype(bf) for b in range(B)]
    wq_s, wk_s, wv_s, wp_s, bq_s, bk_s, bv_s = [], [], [], [], [], [], []
    for hg in range(2):
        sl = slice(512 * hg, 512 * hg + 512)
        wq_s.append(np.ascontiguousarray(WqT[:, sl]).astype(bf))
        wk_s.append(np.ascontiguousarray(WkT[:, sl]).astype(bf))
        wv_s.append(np.ascontiguousarray(WvT[:, sl]).astype(bf))
        wp_s.append(np.ascontiguousarray(WpT[sl, :]).astype(bf))
        bq_s.append(np.ascontiguousarray(bq[sl].reshape(4, 128).T))
        bk_s.append(np.ascontiguousarray(bk[sl].reshape(4, 128).T))
        bv_s.append(np.ascontiguousarray(bv[sl].reshape(1, 512)).astype(bf))

    in_maps = []
    for core in range(NCORES):
        b, hg = core // 2, core % 2
        in_maps.append({
            "xT": xTs[b],
            "wqT": wq_s[hg], "wkT": wk_s[hg], "wvT": wv_s[hg],
            "wpT": wp_s[hg],
            "bq2": bq_s[hg], "bk2": bk_s[hg], "bv2": bv_s[hg],
            "mask": mask,
        })
    return in_maps


def kernel(x, Wq, bq, Wk, bk, Wv, bv, Wp, bp, **_ignored):
    global last_result
    bp = np.asarray(bp, dtype=np.float32)
    in_maps = prep_in_maps(x, Wq, bq, Wk, bk, Wv, bv, Wp, bp)

    if "nc" not in _compiled:
        _compiled["nc"] = _build()
    nc = _compiled["nc"]

    last_result = bass_utils.run_bass_kernel_spmd(
        nc, in_maps, core_ids=list(range(NCORES)))

    out = np.empty((B, T, C), dtype=np.float32)
    for b in range(B):
        out[b] = last_result.results[2 * b]["out"]
        out[b] += last_result.results[2 * b + 1]["out"]
    out += bp[None, None, :]
    return out


# revision 8
# speedup vs baseline: 1.1337x; 1.1337x over previous
"""Causal self-attention (B=4, T=2048, C=1024, H=16) on 8 trn2 NeuronCores.

Sharding: core c -> (batch b = c//2, head-group hg = c%2). Each core computes
q/k/v projections for its 8 heads only (no duplicated K/V work), runs full
causal attention for those heads over all T=2048 queries, and produces a
PARTIAL output projection (contracting its 512 of 1024 y-dims against the
matching Wp rows). The host sums the two partials per batch and adds the
output bias. All cores run an identical SPMD program.

Device pipeline (bf16 matmuls, fp32 PSUM):
  - Warm-up matmuls run during the initial input DMA so the PE clock gate
    (HAM) is released before real work arrives; inputs stream on two DMA
    queues (sync + gpsimd).
  - qT/kT projections in transposed layout [d, t]; v in natural layout
    [t, d] + ones column per head (AV matmul then also yields softmax Z).
  - Attention per head-pair as one flat software-pipelined stream over
    (J, kt) steps: S^T = K Q^T row-packed (tile_position), exp on the scalar
    engine straight out of PSUM for 2/3 of key tiles and as a one-instruction
    Schraudolph bit-trick exp on the vector engine (f32 -> int16 bits
    reinterpreted as bf16) for the remaining 1/3, causal diagonal via
    multiplicative bf16 masks on the gpsimd engine, AV accumulated over key
    tiles in PSUM with 128-granular causal trimming. The AV for step i is
    emitted after step i+1's S/exp so the tensor engine never waits on exp;
    the pipeline runs across J-block boundaries.
  - Projections for later head pairs and deferred softmax normalization are
    interleaved into earlier attention loops; the output projection is
    interleaved per-J into the LAST pair's attention as soon as that J's
    rows are normalized. Partial [2048, 1024] f32 output DMAs alternate
    between two queues.
"""

import numpy as np
import ml_dtypes
from contextlib import ExitStack

import concourse.bass as bass
import concourse.bacc as bacc
import concourse.mybir as mybir
import concourse.tile as tile
from concourse import bass_utils

B, T, C, H = 4, 2048, 1024, 16
HD = C // H            # 64
NCORES = 8
HPC = H // 2           # 8 heads per core
NCH = C // 128         # 8 contraction chunks of x
SCALE = 1.0 / float(np.sqrt(HD))
EXP_A = float(128.0 / np.log(2.0)) * SCALE   # Schraudolph scale (into bf16 bits)
EXP_B = float(16256.0 - 128.0 * 0.0575)      # Schraudolph offset (mean-zero)

bf16 = mybir.dt.bfloat16
f32 = mybir.dt.float32
i16 = mybir.dt.int16
AF = mybir.ActivationFunctionType
ALU = mybir.AluOpType

_compiled = {}
last_result = None  # BassKernelResults of the most recent run (for test harness)


def _build():
    nc = bacc.Bacc("TRN2", target_bir_lowering=False, debug=False,
                   num_devices=NCORES)

    xT_d = nc.dram_tensor("xT", [C, T], bf16, kind="ExternalInput")
    wqT_d = nc.dram_tensor("wqT", [C, 512], bf16, kind="ExternalInput")
    wkT_d = nc.dram_tensor("wkT", [C, 512], bf16, kind="ExternalInput")
    wvT_d = nc.dram_tensor("wvT", [C, 512], bf16, kind="ExternalInput")
    wpT_d = nc.dram_tensor("wpT", [512, C], bf16, kind="ExternalInput")
    bq_d = nc.dram_tensor("bq2", [128, 4], f32, kind="ExternalInput")
    bk_d = nc.dram_tensor("bk2", [128, 4], f32, kind="ExternalInput")
    bv_d = nc.dram_tensor("bv2", [1, 512], bf16, kind="ExternalInput")
    mask_d = nc.dram_tensor("mask", [512, 512], bf16, kind="ExternalInput")
    out_d = nc.dram_tensor("out", [T, C], f32, kind="ExternalOutput")

    xT_v = xT_d.ap().rearrange("(a p) t -> a p t", p=128)
    wq_v = wqT_d.ap().rearrange("(a p) o -> a p o", p=128)
    wk_v = wkT_d.ap().rearrange("(a p) o -> a p o", p=128)
    wv_v = wvT_d.ap().rearrange("(a p) o -> a p o", p=128)
    wp_v = wpT_d.ap().rearrange("(a p) o -> a p o", p=128)
    mask_v = mask_d.ap().rearrange("(a p) i -> a p i", p=128)

    with tile.TileContext(nc) as tc, ExitStack() as ctx:
        persist = ctx.enter_context(tc.tile_pool(name="persist", bufs=1))
        pp = ctx.enter_context(tc.tile_pool(name="pp", bufs=2, space="PSUM"))
        spool = ctx.enter_context(tc.tile_pool(name="spool", bufs=2,
                                               space="PSUM"))
        opool = ctx.enter_context(tc.tile_pool(name="opool", bufs=1,
                                               space="PSUM"))
        ppool = ctx.enter_context(tc.tile_pool(name="ppool", bufs=3))
        outp = ctx.enter_context(tc.tile_pool(name="outp", bufs=3))

        xT_sb = persist.tile([128, NCH, T], bf16)
        qT_sb = persist.tile([128, 4, T], bf16)
        kT_sb = persist.tile([128, 4, T], bf16)
        v_sb = persist.tile([128, 16, HPC, HD + 1], bf16)
        yT_sb = persist.tile([128, 4, T], bf16)
        wq_sb = persist.tile([128, NCH, 512], bf16)
        wk_sb = persist.tile([128, NCH, 512], bf16)
        wv_sb = persist.tile([128, NCH, 512], bf16)
        wp_sb = persist.tile([128, 4, C], bf16)
        bq_sb = persist.tile([128, 4], f32)
        bk_sb = persist.tile([128, 4], f32)
        bv_sb = persist.tile([1, 512], bf16)
        mask_sb = persist.tile([128, 4, 512], bf16)
        zst = persist.tile([128, HPC, 512], bf16)   # Z at row 32J, plane h
        ones_m = persist.tile([1, 128], bf16)    # v-bias broadcast matmul
        ones_r = persist.tile([128, 64], bf16)   # 1/Z broadcast matmul
        warm_w = persist.tile([128, 512], bf16)  # HAM warm-up fodder

        nc.vector.memset(ones_m[:], 1.0)
        nc.vector.memset(ones_r[:], 1.0)
        nc.vector.memset(warm_w[:], 0.125)
        nc.vector.memset(v_sb[:, :, :, HD:HD + 1], 1.0)  # aug ones column
        nc.vector.memset(zst[:], 1.0)

        # input DMAs on two queues: sync carries xT, gpsimd the weights
        for c in range(NCH):
            nc.sync.dma_start(xT_sb[:, c, :], xT_v[c])
            nc.gpsimd.dma_start(wq_sb[:, c, :], wq_v[c])
            nc.gpsimd.dma_start(wk_sb[:, c, :], wk_v[c])
        nc.sync.dma_start(bq_sb[:], bq_d.ap())
        nc.sync.dma_start(bk_sb[:], bk_d.ap())
        nc.sync.dma_start(bv_sb[:], bv_d.ap())
        for c in range(NCH):
            nc.gpsimd.dma_start(wv_sb[:, c, :], wv_v[c])
        for m in range(4):
            nc.gpsimd.dma_start(mask_sb[:, m, :], mask_v[m])
        for c in range(4):
            nc.gpsimd.dma_start(wp_sb[:, c, :], wp_v[c])

        # PE warm-up during the input DMA window (junk matmuls)
        for _ in range(16):
            ps = pp.tile([128, 512], f32, tag="pp")
            nc.tensor.matmul(ps[:], warm_w[:, 0:128], warm_w[:],
                             start=True, stop=True)

        # ---------------- emission helpers ----------------
        def qk_unit(hp, tn, t4):
            w_sb, b_sb, dst = (wq_sb, bq_sb, qT_sb) if tn == 0 else \
                              (wk_sb, bk_sb, kT_sb)
            ps = pp.tile([128, 512], f32, tag="pp")
            for c in range(NCH):
                nc.tensor.matmul(
                    ps[:], w_sb[:, c, 128 * hp:128 * hp + 128],
                    xT_sb[:, c, 512 * t4:512 * t4 + 512],
                    start=(c == 0), stop=(c == NCH - 1))
            nc.vector.tensor_scalar_add(
                dst[:, hp, 512 * t4:512 * t4 + 512], ps[:], b_sb[:, hp:hp + 1])

        def v_unit(r):
            ps = pp.tile([128, 512], f32, tag="pp")
            for c in range(NCH):
                nc.tensor.matmul(
                    ps[:], xT_sb[:, c, 128 * r:128 * r + 128], wv_sb[:, c, :],
                    start=(c == 0), stop=False)
            nc.tensor.matmul(ps[:], ones_m[:], bv_sb[:],
                             start=False, stop=True)
            nc.vector.tensor_copy(
                v_sb[:, r, :, 0:HD],
                ps[:].rearrange("p (h e) -> p h e", e=HD))

        def norm_unit(h, J):
            b = 32 * J
            qs = slice(512 * J, 512 * J + 512)
            bp1 = pp.tile([64, 512], f32, tag="pp")
            nc.tensor.matmul(bp1[:], ones_r[b:b + 1, :], zst[b:b + 1, h, :],
                             tile_position=(b, 0))
            nc.vector.reciprocal_approx_fast(bp1[:], bp1[:])
            pr = 64 * (h % 2)
            nc.vector.tensor_mul(yT_sb[pr:pr + 64, h // 2, qs],
                                 yT_sb[pr:pr + 64, h // 2, qs], bp1[:])

        def p_unit(qt, co):
            ps = pp.tile([128, 512], f32, tag="pp")
            for c2 in range(4):
                nc.tensor.matmul(
                    ps[:], yT_sb[:, c2, 128 * qt:128 * qt + 128],
                    wp_sb[:, c2, 512 * co:512 * co + 512],
                    start=(c2 == 0), stop=(c2 == 3))
            ot = outp.tile([128, 512], f32, tag="ot")
            if co == 0:
                nc.vector.tensor_copy(ot[:], ps[:])
            else:
                nc.scalar.activation(ot[:], ps[:], AF.Copy)
            eng = nc.sync if (qt + co) % 2 == 0 else nc.gpsimd
            eng.dma_start(
                out_d.ap()[128 * qt:128 * qt + 128,
                           512 * co:512 * co + 512], ot[:])

        def attention_pair(hp, tasks, jhook=None):
            steps = [(J, kt) for J in range(4) for kt in range(4 * (J + 1))]
            n = len(steps)
            state = {"emitted": 0, "cur": None, "pend": None}

            def drain_to(k):
                while state["emitted"] < min(k, len(tasks)):
                    tasks[state["emitted"]]()
                    state["emitted"] += 1

            def flush():
                J, kt, p2, i0 = state["pend"]
                state["pend"] = None
                E = 4 * (J + 1)
                if kt == 0:
                    oA = opool.tile([HD + 1, 512], f32, tag="oA", name="oA")
                    oB = opool.tile([HD + 1, 512], f32, tag="oB", name="oB")
                    state["cur"] = (oA, oB)
                oA, oB = state["cur"]
                last = (kt == E - 1)
                nc.tensor.matmul(oA[:, i0:512], v_sb[:, kt, 2 * hp, :],
                                 p2[:, i0:512], start=(kt == 0), stop=last)
                nc.tensor.matmul(oB[:, i0:512], v_sb[:, kt, 2 * hp + 1, :],
                                 p2[:, 512 + i0:1024],
                                 start=(kt == 0), stop=last)
                if last:
                    qs = slice(512 * J, 512 * J + 512)
                    nc.vector.tensor_copy(yT_sb[0:64, hp, qs], oA[0:HD, :])
                    nc.vector.tensor_copy(yT_sb[64:128, hp, qs], oB[0:HD, :])
                    nc.vector.tensor_copy(zst[32 * J:32 * J + 1, 2 * hp, :],
                                          oA[HD:HD + 1, :])
                    nc.vector.tensor_copy(
                        zst[32 * J:32 * J + 1, 2 * hp + 1, :],
                        oB[HD:HD + 1, :])
                    if jhook is not None:
                        jhook(J, drain_to)

            for idx, (J, kt) in enumerate(steps):
                ks = slice(128 * kt, 128 * kt + 128)
                i0 = 128 * (kt - 4 * J) if kt >= 4 * J else 0
                s2 = spool.tile([128, 1024], f32, tag="s2")
                nc.tensor.matmul(s2[:, i0:512], kT_sb[0:64, hp, ks],
                                 qT_sb[0:64, hp, 512 * J + i0:512 * J + 512],
                                 tile_position=(0, 0))
                nc.tensor.matmul(s2[:, 512 + i0:1024], kT_sb[64:128, hp, ks],
                                 qT_sb[64:128, hp,
                                       512 * J + i0:512 * J + 512],
                                 tile_position=(64, 0))
                p2 = ppool.tile([128, 1024], bf16, tag="p2")
                s2v = s2[:].rearrange("p (h q) -> p h q", q=512)
                p2v = p2[:].rearrange("p (h q) -> p h q", q=512)
                if kt % 3 == 1:   # Schraudolph exp on the vector engine
                    nc.vector.tensor_scalar(
                        p2v[:, :, i0:512].bitcast(i16), s2v[:, :, i0:512],
                        EXP_A, EXP_B, ALU.mult, ALU.add)
                else:             # spline exp on the scalar engine
                    nc.scalar.activation(p2v[:, :, i0:512], s2v[:, :, i0:512],
                                         AF.Exp, scale=SCALE)
                if kt >= 4 * J:  # diagonal block: causal mask
                    # only columns [i0, i0+128) straddle the triangle; the
                    # rest of the tile is all-ones
                    m = kt - 4 * J
                    ie = min(i0 + 128, 512)
                    nc.vector.tensor_mul(p2[:, i0:ie], p2[:, i0:ie],
                                         mask_sb[:, m, i0:ie])
                    nc.vector.tensor_mul(p2[:, 512 + i0:512 + ie],
                                         p2[:, 512 + i0:512 + ie],
                                         mask_sb[:, m, i0:ie])
                drain_to(len(tasks) * (idx + 1) // n)
                if state["pend"] is not None:
                    flush()
                state["pend"] = (J, kt, p2, i0)
            flush()
            drain_to(len(tasks))
            return drain_to

        # ---------------- schedule ----------------
        # prologue: projections for pair 0, first half of v
        for tn in range(2):
            for t4 in range(4):
                qk_unit(0, tn, t4)
        for r in range(8):
            v_unit(r)

        def mk_v(r):
            return lambda: v_unit(r)

        def mk_qk(hp, tn, t4):
            return lambda: qk_unit(hp, tn, t4)

        def mk_norm(h, J):
            return lambda: norm_unit(h, J)

        # pair 0: rest of v + projections for pair 1
        attention_pair(0, [mk_v(r) for r in range(8, 16)] +
                       [mk_qk(1, tn, t4) for tn in range(2)
                        for t4 in range(4)])
        # pair 1: projections for pair 2, then pair-0 normalization
        attention_pair(1, [mk_qk(2, tn, t4) for tn in range(2)
                           for t4 in range(4)] +
                       [mk_norm(h, J) for h in (0, 1) for J in range(4)])
        # pair 2: projections for pair 3, then pair-1 normalization
        attention_pair(2, [mk_qk(3, tn, t4) for tn in range(2)
                           for t4 in range(4)] +
                       [mk_norm(h, J) for h in (2, 3) for J in range(4)])

        # pair 3: pair-2 normalization tasks (J-major so the jhook can
        # force-drain them per J), plus per-J appended work: pair-3
        # normalization and the output projection of that J's token rows
        p3_tasks = [mk_norm(h, J) for J in range(4) for h in (4, 5)]

        def mk_p(qt, co):
            return lambda: p_unit(qt, co)

        def jhook3(J, drain_to):
            drain_to(2 * J + 2)          # norm(4, J), norm(5, J) done
            p3_tasks.append(mk_norm(6, J))
            p3_tasks.append(mk_norm(7, J))
            for qt in range(4 * J, 4 * J + 4):
                for co in range(2):
                    p3_tasks.append(mk_p(qt, co))

        attention_pair(3, p3_tasks, jhook=jhook3)

    nc.compile()
    return nc


def prep_in_maps(x, Wq, bq, Wk, bk, Wv, bv, Wp, bp):
    x = np.asarray(x, dtype=np.float32)
    Wq = np.asarray(Wq, dtype=np.float32)
    Wk = np.asarray(Wk, dtype=np.float32)
    Wv = np.asarray(Wv, dtype=np.float32)
    Wp = np.asarray(Wp, dtype=np.float32)
    bq = np.asarray(bq, dtype=np.float32)
    bk = np.asarray(bk, dtype=np.float32)
    bv = np.asarray(bv, dtype=np.float32)

    bf = ml_dtypes.bfloat16
    WqT, WkT, WvT, WpT = Wq.T, Wk.T, Wv.T, Wp.T

    kk = np.arange(128)[:, None]
    qq = np.arange(512)[None, :]
    mask = np.ascontiguousarray(np.concatenate(
        [(128 * m + kk <= qq) for m in range(4)], axis=0).astype(bf))

    xTs = [np.ascontiguousarray(x[b].T).astype(bf) for b in range(B)]
    wq_s, wk_s, wv_s, wp_s, bq_s, bk_s, bv_s = [], [], [], [], [], [], []
    for hg in range(2):
        sl = slice(512 * hg, 512 * hg + 512)
        wq_s.append(np.ascontiguousarray(WqT[:, sl]).astype(bf))
        wk_s.append(np.ascontiguousarray(WkT[:, sl]).astype(bf))
        wv_s.append(np.ascontiguousarray(WvT[:, sl]).astype(bf))
        wp_s.append(np.ascontiguousarray(WpT[sl, :]).astype(bf))
        bq_s.append(np.ascontiguousarray(bq[sl].reshape(4, 128).T))
        bk_s.append(np.ascontiguousarray(bk[sl].reshape(4, 128).T))
        bv_s.append(np.ascontiguousarray(bv[sl].reshape(1, 512)).astype(bf))

    in_maps = []
    for core in range(NCORES):
        b, hg = core // 2, core % 2
        in_maps.append({
            "xT": xTs[b],
            "wqT": wq_s[hg], "wkT": wk_s[hg], "wvT": wv_s[hg],
            "wpT": wp_s[hg],
            "bq2": bq_s[hg], "bk2": bk_s[hg], "bv2": bv_s[hg],
            "mask": mask,
        })
    return in_maps


def kernel(x, Wq, bq, Wk, bk, Wv, bv, Wp, bp, **_ignored):
    global last_result
    bp = np.asarray(bp, dtype=np.float32)
    in_maps = prep_in_maps(x, Wq, bq, Wk, bk, Wv, bv, Wp, bp)

    if "nc" not in _compiled:
        _compiled["nc"] = _build()
    nc = _compiled["nc"]

    last_result = bass_utils.run_bass_kernel_spmd(
        nc, in_maps, core_ids=list(range(NCORES)))

    out = np.empty((B, T, C), dtype=np.float32)
    for b in range(B):
        out[b] = last_result.results[2 * b]["out"]
        out[b] += last_result.results[2 * b + 1]["out"]
    out += bp[None, None, :]
    return out


# revision 15
# speedup vs baseline: 1.2258x; 1.0812x over previous
"""Causal self-attention (B=4, T=2048, C=1024, H=16) on 8 trn2 NeuronCores.

Sharding: core c -> (batch b = c//2, head-group hg = c%2). Each core computes
q/k/v projections for its 8 heads only (no duplicated K/V work), runs full
causal attention for those heads over all T=2048 queries, and produces a
PARTIAL output projection (contracting its 512 of 1024 y-dims against the
matching Wp rows). The host sums the two partials per batch and adds the
output bias. All cores run an identical SPMD program.

Device pipeline (bf16 matmuls, fp32 PSUM):
  - Warm-up matmuls run during the initial input DMA so the PE clock gate
    (HAM) is released before real work arrives; inputs stream on two DMA
    queues (sync + gpsimd).
  - qT/kT projections in transposed layout [d, t]; v in natural layout
    [t, d] + ones column per head (AV matmul then also yields softmax Z).
  - Attention per head-pair as one flat software-pipelined stream over
    (J, kt) steps: S^T = K Q^T row-packed (tile_position), exp on the scalar
    engine straight out of PSUM for 2/3 of key tiles and as a one-instruction
    Schraudolph bit-trick exp on the vector engine (f32 -> int16 bits
    reinterpreted as bf16) for the remaining 1/3, causal diagonal via
    multiplicative bf16 masks on the gpsimd engine, AV accumulated over key
    tiles in PSUM with 128-granular causal trimming. The AV for step i is
    emitted after step i+1's S/exp so the tensor engine never waits on exp;
    the pipeline runs across J-block boundaries.
  - Projections for later head pairs and deferred softmax normalization are
    interleaved into earlier attention loops; the output projection is
    interleaved per-J into the LAST pair's attention as soon as that J's
    rows are normalized. Partial [2048, 1024] f32 output DMAs alternate
    between two queues.
"""

import numpy as np
import ml_dtypes
from contextlib import ExitStack

import concourse.bass as bass
import concourse.bacc as bacc
import concourse.mybir as mybir
import concourse.tile as tile
from concourse import bass_utils

B, T, C, H = 4, 2048, 1024, 16
HD = C // H            # 64
NCORES = 8
HPC = H // 2           # 8 heads per core
NCH = C // 128         # 8 contraction chunks of x
SCALE = 1.0 / float(np.sqrt(HD))
EXP_A = float(128.0 / np.log(2.0)) * SCALE   # Schraudolph scale (into bf16 bits)
EXP_B = float(16256.0 - 128.0 * 0.0575)      # Schraudolph offset (mean-zero)

bf16 = mybir.dt.bfloat16
f32 = mybir.dt.float32
i16 = mybir.dt.int16
AF = mybir.ActivationFunctionType
ALU = mybir.AluOpType

_compiled = {}
last_result = None  # BassKernelResults of the most recent run (for test harness)


def _build():
    nc = bacc.Bacc("TRN2", target_bir_lowering=False, debug=False,
                   num_devices=NCORES)

    xT_d = nc.dram_tensor("xT", [C, T], bf16, kind="ExternalInput")
    wqT_d = nc.dram_tensor("wqT", [C, 512], bf16, kind="ExternalInput")
    wkT_d = nc.dram_tensor("wkT", [C, 512], bf16, kind="ExternalInput")
    wvT_d = nc.dram_tensor("wvT", [C, 512], bf16, kind="ExternalInput")
    wpT_d = nc.dram_tensor("wpT", [512, C], bf16, kind="ExternalInput")
    bq_d = nc.dram_tensor("bq2", [128, 4], f32, kind="ExternalInput")
    bk_d = nc.dram_tensor("bk2", [128, 4], f32, kind="ExternalInput")
    bv_d = nc.dram_tensor("bv2", [1, 512], bf16, kind="ExternalInput")
    mask_d = nc.dram_tensor("mask", [512, 512], bf16, kind="ExternalInput")
    out_d = nc.dram_tensor("out", [T, C], f32, kind="ExternalOutput")

    xT_v = xT_d.ap().rearrange("(a p) t -> a p t", p=128)
    wq_v = wqT_d.ap().rearrange("(a p) o -> a p o", p=128)
    wk_v = wkT_d.ap().rearrange("(a p) o -> a p o", p=128)
    wv_v = wvT_d.ap().rearrange("(a p) o -> a p o", p=128)
    wp_v = wpT_d.ap().rearrange("(a p) o -> a p o", p=128)
    mask_v = mask_d.ap().rearrange("(a p) i -> a p i", p=128)

    with tile.TileContext(nc) as tc, ExitStack() as ctx:
        persist = ctx.enter_context(tc.tile_pool(name="persist", bufs=1))
        pp = ctx.enter_context(tc.tile_pool(name="pp", bufs=2, space="PSUM"))
        spool = ctx.enter_context(tc.tile_pool(name="spool", bufs=2,
                                               space="PSUM"))
        opool = ctx.enter_context(tc.tile_pool(name="opool", bufs=1,
                                               space="PSUM"))
        ppool = ctx.enter_context(tc.tile_pool(name="ppool", bufs=3))
        outp = ctx.enter_context(tc.tile_pool(name="outp", bufs=3))

        xT_sb = persist.tile([128, NCH, T], bf16)
        qT_sb = persist.tile([128, 4, T], bf16)
        kT_sb = persist.tile([128, 4, T], bf16)
        v_sb = persist.tile([128, 16, HPC, HD + 1], bf16)
        yT_sb = persist.tile([128, 4, T], bf16)
        wq_sb = persist.tile([128, NCH, 512], bf16)
        wk_sb = persist.tile([128, NCH, 512], bf16)
        wv_sb = persist.tile([128, NCH, 512], bf16)
        wp_sb = persist.tile([128, 4, C], bf16)
        bq_sb = persist.tile([128, 4], f32)
        bk_sb = persist.tile([128, 4], f32)
        bv_sb = persist.tile([1, 512], bf16)
        mask_sb = persist.tile([128, 4, 512], bf16)
        zst = persist.tile([128, HPC, 512], bf16)   # Z at row 32J, plane h
        ones_m = persist.tile([1, 128], bf16)    # v-bias broadcast matmul
        ones_r = persist.tile([128, 64], bf16)   # 1/Z broadcast matmul
        warm_w = persist.tile([128, 512], bf16)  # HAM warm-up fodder

        nc.vector.memset(ones_m[:], 1.0)
        nc.vector.memset(ones_r[:], 1.0)
        nc.vector.memset(warm_w[:], 0.125)
        nc.vector.memset(v_sb[:, :, :, HD:HD + 1], 1.0)  # aug ones column
        nc.vector.memset(zst[:], 1.0)

        # input DMAs on two queues: sync carries xT, gpsimd the weights
        for c in range(NCH):
            nc.sync.dma_start(xT_sb[:, c, :], xT_v[c])
            nc.gpsimd.dma_start(wq_sb[:, c, :], wq_v[c])
            nc.gpsimd.dma_start(wk_sb[:, c, :], wk_v[c])
        nc.sync.dma_start(bq_sb[:], bq_d.ap())
        nc.sync.dma_start(bk_sb[:], bk_d.ap())
        nc.sync.dma_start(bv_sb[:], bv_d.ap())
        for c in range(NCH):
            nc.gpsimd.dma_start(wv_sb[:, c, :], wv_v[c])
        for m in range(4):
            nc.gpsimd.dma_start(mask_sb[:, m, :], mask_v[m])
        for c in range(4):
            nc.gpsimd.dma_start(wp_sb[:, c, :], wp_v[c])

        # PE warm-up during the input DMA window (junk matmuls)
        for _ in range(16):
            ps = pp.tile([128, 512], f32, tag="pp")
            nc.tensor.matmul(ps[:], warm_w[:, 0:128], warm_w[:],
                             start=True, stop=True)

        # ---------------- emission helpers ----------------
        def qk_unit(hp, tn, t4):
            w_sb, b_sb, dst = (wq_sb, bq_sb, qT_sb) if tn == 0 else \
                              (wk_sb, bk_sb, kT_sb)
            ps = pp.tile([128, 512], f32, tag="pp")
            for c in range(NCH):
                nc.tensor.matmul(
                    ps[:], w_sb[:, c, 128 * hp:128 * hp + 128],
                    xT_sb[:, c, 512 * t4:512 * t4 + 512],
                    start=(c == 0), stop=(c == NCH - 1))
            nc.vector.tensor_scalar_add(
                dst[:, hp, 512 * t4:512 * t4 + 512], ps[:], b_sb[:, hp:hp + 1])

        def v_unit(r):
            ps = pp.tile([128, 512], f32, tag="pp")
            for c in range(NCH):
                nc.tensor.matmul(
                    ps[:], xT_sb[:, c, 128 * r:128 * r + 128], wv_sb[:, c, :],
                    start=(c == 0), stop=False)
            nc.tensor.matmul(ps[:], ones_m[:], bv_sb[:],
                             start=False, stop=True)
            nc.vector.tensor_copy(
                v_sb[:, r, :, 0:HD],
                ps[:].rearrange("p (h e) -> p h e", e=HD))

        def norm_unit(hp, J):
            # broadcast both heads' Z of block J into one PSUM bank
            # (row 32J stationary, col positions 0/64), then one reciprocal
            # and one multiply normalize the whole [128, 512] yT slice
            b = 32 * J
            qs = slice(512 * J, 512 * J + 512)
            bp2 = pp.tile([128, 512], f32, tag="pp")
            nc.tensor.matmul(bp2[0:64, :], ones_r[b:b + 1, :],
                             zst[b:b + 1, 2 * hp, :], tile_position=(b, 0))
            nc.tensor.matmul(bp2[64:128, :], ones_r[b:b + 1, :],
                             zst[b:b + 1, 2 * hp + 1, :],
                             tile_position=(b, 64))
            nc.vector.reciprocal_approx_fast(bp2[:], bp2[:])
            nc.vector.tensor_mul(yT_sb[:, hp, qs], yT_sb[:, hp, qs], bp2[:])

        def p_unit(qt, co):
            ps = pp.tile([128, 512], f32, tag="pp")
            for c2 in range(4):
                nc.tensor.matmul(
                    ps[:], yT_sb[:, c2, 128 * qt:128 * qt + 128],
                    wp_sb[:, c2, 512 * co:512 * co + 512],
                    start=(c2 == 0), stop=(c2 == 3))
            ot = outp.tile([128, 512], f32, tag="ot")
            if co == 0:
                nc.vector.tensor_copy(ot[:], ps[:])
            else:
                nc.scalar.activation(ot[:], ps[:], AF.Copy)
            eng = nc.sync if (qt + co) % 2 == 0 else nc.gpsimd
            eng.dma_start(
                out_d.ap()[128 * qt:128 * qt + 128,
                           512 * co:512 * co + 512], ot[:])

        def attention_pair(hp, tasks, jhook=None):
            steps = [(J, kt) for J in range(4) for kt in range(4 * (J + 1))]
            n = len(steps)
            state = {"emitted": 0, "cur": None, "pend": None}

            def drain_to(k):
                while state["emitted"] < min(k, len(tasks)):
                    tasks[state["emitted"]]()
                    state["emitted"] += 1

            def flush():
                J, kt, p2, i0 = state["pend"]
                state["pend"] = None
                E = 4 * (J + 1)
                if kt == 0:
                    oA = opool.tile([HD + 1, 512], f32, tag="oA", name="oA")
                    oB = opool.tile([HD + 1, 512], f32, tag="oB", name="oB")
                    state["cur"] = (oA, oB)
                oA, oB = state["cur"]
                last = (kt == E - 1)
                nc.tensor.matmul(oA[:, i0:512], v_sb[:, kt, 2 * hp, :],
                                 p2[:, i0:512], start=(kt == 0), stop=last)
                nc.tensor.matmul(oB[:, i0:512], v_sb[:, kt, 2 * hp + 1, :],
                                 p2[:, 512 + i0:1024],
                                 start=(kt == 0), stop=last)
                if last:
                    qs = slice(512 * J, 512 * J + 512)
                    nc.scalar.activation(yT_sb[0:64, hp, qs], oA[0:HD, :],
                                         AF.Copy)
                    nc.vector.tensor_copy(yT_sb[64:128, hp, qs], oB[0:HD, :])
                    b = 32 * J
                    nc.vector.tensor_copy(zst[b:b + 1, 2 * hp, :],
                                          oA[HD:HD + 1, :])
                    nc.vector.tensor_copy(zst[b:b + 1, 2 * hp + 1, :],
                                          oB[HD:HD + 1, :])
                    if jhook is not None:
                        jhook(J, drain_to)

            for idx, (J, kt) in enumerate(steps):
                ks = slice(128 * kt, 128 * kt + 128)
                i0 = 128 * (kt - 4 * J) if kt >= 4 * J else 0
                s2 = spool.tile([128, 1024], f32, tag="s2")
                nc.tensor.matmul(s2[:, i0:512], kT_sb[0:64, hp, ks],
                                 qT_sb[0:64, hp, 512 * J + i0:512 * J + 512],
                                 tile_position=(0, 0))
                nc.tensor.matmul(s2[:, 512 + i0:1024], kT_sb[64:128, hp, ks],
                                 qT_sb[64:128, hp,
                                       512 * J + i0:512 * J + 512],
                                 tile_position=(64, 0))
                p2 = ppool.tile([128, 1024], bf16, tag="p2")
                s2v = s2[:].rearrange("p (h q) -> p h q", q=512)
                p2v = p2[:].rearrange("p (h q) -> p h q", q=512)
                if kt % 3 == 1:   # Schraudolph exp on the vector engine
                    nc.vector.tensor_scalar(
                        p2v[:, :, i0:512].bitcast(i16), s2v[:, :, i0:512],
                        EXP_A, EXP_B, ALU.mult, ALU.add)
                else:             # spline exp on the scalar engine
                    nc.scalar.activation(p2v[:, :, i0:512], s2v[:, :, i0:512],
                                         AF.Exp, scale=SCALE)
                if kt >= 4 * J:  # diagonal block: causal mask
                    # only columns [i0, i0+128) straddle the triangle; the
                    # rest of the tile is all-ones
                    m = kt - 4 * J
                    ie = min(i0 + 128, 512)
                    nc.vector.tensor_mul(p2[:, i0:ie], p2[:, i0:ie],
                                         mask_sb[:, m, i0:ie])
                    nc.vector.tensor_mul(p2[:, 512 + i0:512 + ie],
                                         p2[:, 512 + i0:512 + ie],
                                         mask_sb[:, m, i0:ie])
                drain_to(len(tasks) * (idx + 1) // n)
                if state["pend"] is not None:
                    flush()
                state["pend"] = (J, kt, p2, i0)
            flush()
            drain_to(len(tasks))
            return drain_to

        # ---------------- schedule ----------------
        # prologue: projections for pair 0, first half of v
        for tn in range(2):
            for t4 in range(4):
                qk_unit(0, tn, t4)
        for r in range(8):
            v_unit(r)

        def mk_v(r):
            return lambda: v_unit(r)

        def mk_qk(hp, tn, t4):
            return lambda: qk_unit(hp, tn, t4)

        def mk_norm(hp, J):
            return lambda: norm_unit(hp, J)

        # pair 0: rest of v + projections for pair 1
        attention_pair(0, [mk_v(r) for r in range(8, 16)] +
                       [mk_qk(1, tn, t4) for tn in range(2)
                        for t4 in range(4)])
        # pair 1: projections for pair 2, then pair-0 normalization
        attention_pair(1, [mk_qk(2, tn, t4) for tn in range(2)
                           for t4 in range(4)] +
                       [mk_norm(0, J) for J in range(4)])
        # pair 2: projections for pair 3, then pair-1 normalization
        attention_pair(2, [mk_qk(3, tn, t4) for tn in range(2)
                           for t4 in range(4)] +
                       [mk_norm(1, J) for J in range(4)])

        # pair 3: pair-2 normalization tasks (J-major so the jhook can
        # force-drain them per J), plus per-J appended work: pair-3
        # normalization and the output projection of that J's token rows
        p3_tasks = [mk_norm(2, J) for J in range(4)]

        def mk_p(qt, co):
            return lambda: p_unit(qt, co)

        def jhook3(J, drain_to):
            drain_to(J + 1)              # norm(pair 2, J) done
            p3_tasks.append(mk_norm(3, J))
            for qt in range(4 * J, 4 * J + 4):
                for co in range(2):
                    p3_tasks.append(mk_p(qt, co))

        attention_pair(3, p3_tasks, jhook=jhook3)

    nc.compile()
    return nc


def prep_in_maps(x, Wq, bq, Wk, bk, Wv, bv, Wp, bp):
    x = np.asarray(x, dtype=np.float32)
    Wq = np.asarray(Wq, dtype=np.float32)
    Wk = np.asarray(Wk, dtype=np.float32)
    Wv = np.asarray(Wv, dtype=np.float32)
    Wp = np.asarray(Wp, dtype=np.float32)
    bq = np.asarray(bq, dtype=np.float32)
    bk = np.asarray(bk, dtype=np.float32)
    bv = np.asarray(bv, dtype=np.float32)

    bf = ml_dtypes.bfloat16
    WqT, WkT, WvT, WpT = Wq.T, Wk.T, Wv.T, Wp.T

    kk = np.arange(128)[:, None]
    qq = np.arange(512)[None, :]
    mask = np.ascontiguousarray(np.concatenate(
        [(128 * m + kk <= qq) for m in range(4)], axis=0).astype(bf))

    xTs = [np.ascontiguousarray(x[b].T).astype(bf) for b in range(B)]
    wq_s, wk_s, wv_s, wp_s, bq_s, bk_s, bv_s = [], [], [], [], [], [], []
    for hg in range(2):
        sl = slice(512 * hg, 512 * hg + 512)
        wq_s.append(np.ascontiguousarray(WqT[:, sl]).astype(bf))
        wk_s.append(np.ascontiguousarray(WkT[:, sl]).astype(bf))
        wv_s.append(np.ascontiguousarray(WvT[:, sl]).astype(bf))
        wp_s.append(np.ascontiguousarray(WpT[sl, :]).astype(bf))
        bq_s.append(np.ascontiguousarray(bq[sl].reshape(4, 128).T))
        bk_s.append(np.ascontiguousarray(bk[sl].reshape(4, 128).T))
        bv_s.append(np.ascontiguousarray(bv[sl].reshape(1, 512)).astype(bf))

    in_maps = []
    for core in range(NCORES):
        b, hg = core // 2, core % 2
        in_maps.append({
            "xT": xTs[b],
            "wqT": wq_s[hg], "wkT": wk_s[hg], "wvT": wv_s[hg],
            "wpT": wp_s[hg],
            "bq2": bq_s[hg], "bk2": bk_s[hg], "bv2": bv_s[hg],
            "mask": mask,
        })
    return in_maps


def kernel(x, Wq, bq, Wk, bk, Wv, bv, Wp, bp, **_ignored):
    global last_result
    bp = np.asarray(bp, dtype=np.float32)
    in_maps = prep_in_maps(x, Wq, bq, Wk, bk, Wv, bv, Wp, bp)

    if "nc" not in _compiled:
        _compiled["nc"] = _build()
    nc = _compiled["nc"]

    last_result = bass_utils.run_bass_kernel_spmd(
        nc, in_maps, core_ids=list(range(NCORES)))

    out = np.empty((B, T, C), dtype=np.float32)
    for b in range(B):
        out[b] = last_result.results[2 * b]["out"]
        out[b] += last_result.results[2 * b + 1]["out"]
    out += bp[None, None, :]
    return out


# revision 18
# speedup vs baseline: 1.2490x; 1.0189x over previous
"""Causal self-attention (B=4, T=2048, C=1024, H=16) on 8 trn2 NeuronCores.

Sharding: core c -> (batch b = c//2, head-group hg = c%2). Each core computes
q/k/v projections for its 8 heads only (no duplicated K/V work), runs full
causal attention for those heads over all T=2048 queries, and produces a
PARTIAL output projection (contracting its 512 of 1024 y-dims against the
matching Wp rows). The host sums the two partials per batch and adds the
output bias. All cores run an identical SPMD program.

Device pipeline (bf16 matmuls, fp32 PSUM):
  - Warm-up matmuls run during the initial input DMA so the PE clock gate
    (HAM) is released before real work arrives; inputs stream on two DMA
    queues (sync + gpsimd).
  - qT/kT projections in transposed layout [d, t]; v in natural layout
    [t, d] + ones column per head (AV matmul then also yields softmax Z).
  - Attention per head-pair as one flat software-pipelined stream over
    (J, kt) steps: S^T = K Q^T row-packed (tile_position), exp on the scalar
    engine straight out of PSUM for 2/3 of key tiles and as a one-instruction
    Schraudolph bit-trick exp on the vector engine (f32 -> int16 bits
    reinterpreted as bf16) for the remaining 1/3, causal diagonal via
    multiplicative bf16 masks on the gpsimd engine, AV accumulated over key
    tiles in PSUM with 128-granular causal trimming. The AV for step i is
    emitted after step i+1's S/exp so the tensor engine never waits on exp;
    the pipeline runs across J-block boundaries.
  - Projections for later head pairs and deferred softmax normalization are
    interleaved into earlier attention loops; the output projection is
    interleaved per-J into the LAST pair's attention as soon as that J's
    rows are normalized. Partial [2048, 1024] f32 output DMAs alternate
    between two queues.
"""

import numpy as np
import ml_dtypes
from contextlib import ExitStack

import concourse.bass as bass
import concourse.bacc as bacc
import concourse.mybir as mybir
import concourse.tile as tile
from concourse import bass_utils

B, T, C, H = 4, 2048, 1024, 16
HD = C // H            # 64
NCORES = 8
HPC = H // 2           # 8 heads per core
NCH = C // 128         # 8 contraction chunks of x
SCALE = 1.0 / float(np.sqrt(HD))
EXP_A = float(128.0 / np.log(2.0)) * SCALE   # Schraudolph scale (into bf16 bits)
EXP_B = float(16256.0 - 128.0 * 0.0575)      # Schraudolph offset (mean-zero)

bf16 = mybir.dt.bfloat16
f32 = mybir.dt.float32
i16 = mybir.dt.int16
AF = mybir.ActivationFunctionType
ALU = mybir.AluOpType

_compiled = {}
last_result = None  # BassKernelResults of the most recent run (for test harness)


def _build():
    nc = bacc.Bacc("TRN2", target_bir_lowering=False, debug=False,
                   num_devices=NCORES)

    xT_d = nc.dram_tensor("xT", [C, T], bf16, kind="ExternalInput")
    wqT_d = nc.dram_tensor("wqT", [C, 512], bf16, kind="ExternalInput")
    wkT_d = nc.dram_tensor("wkT", [C, 512], bf16, kind="ExternalInput")
    wvT_d = nc.dram_tensor("wvT", [C, 512], bf16, kind="ExternalInput")
    wpT_d = nc.dram_tensor("wpT", [512, C], bf16, kind="ExternalInput")
    bq_d = nc.dram_tensor("bq2", [128, 4], f32, kind="ExternalInput")
    bk_d = nc.dram_tensor("bk2", [128, 4], f32, kind="ExternalInput")
    bv_d = nc.dram_tensor("bv2", [1, 512], bf16, kind="ExternalInput")
    mask_d = nc.dram_tensor("mask", [512, 512], bf16, kind="ExternalInput")
    out_d = nc.dram_tensor("out", [T, C], f32, kind="ExternalOutput")

    xT_v = xT_d.ap().rearrange("(a p) t -> a p t", p=128)
    wq_v = wqT_d.ap().rearrange("(a p) o -> a p o", p=128)
    wk_v = wkT_d.ap().rearrange("(a p) o -> a p o", p=128)
    wv_v = wvT_d.ap().rearrange("(a p) o -> a p o", p=128)
    wp_v = wpT_d.ap().rearrange("(a p) o -> a p o", p=128)
    mask_v = mask_d.ap().rearrange("(a p) i -> a p i", p=128)

    with tile.TileContext(nc) as tc, ExitStack() as ctx:
        persist = ctx.enter_context(tc.tile_pool(name="persist", bufs=1))
        pp = ctx.enter_context(tc.tile_pool(name="pp", bufs=2, space="PSUM"))
        spool = ctx.enter_context(tc.tile_pool(name="spool", bufs=2,
                                               space="PSUM"))
        opool = ctx.enter_context(tc.tile_pool(name="opool", bufs=1,
                                               space="PSUM"))
        ppool = ctx.enter_context(tc.tile_pool(name="ppool", bufs=3))
        outp = ctx.enter_context(tc.tile_pool(name="outp", bufs=3))

        xT_sb = persist.tile([128, NCH, T], bf16)
        qT_sb = persist.tile([128, 4, T], bf16)
        kT_sb = persist.tile([128, 4, T], bf16)
        v_sb = persist.tile([128, 16, HPC, HD + 1], bf16)
        yT_sb = persist.tile([128, 4, T], bf16)
        wq_sb = persist.tile([128, NCH, 512], bf16)
        wk_sb = persist.tile([128, NCH, 512], bf16)
        wv_sb = persist.tile([128, NCH, 512], bf16)
        wp_sb = persist.tile([128, 4, C], bf16)
        bq_sb = persist.tile([128, 4], f32)
        bk_sb = persist.tile([128, 4], f32)
        bv_sb = persist.tile([1, 512], bf16)
        mask_sb = persist.tile([128, 4, 512], bf16)
        zst = persist.tile([128, HPC, 512], bf16)   # Z at row 32J, plane h
        ones_m = persist.tile([1, 128], bf16)    # v-bias broadcast matmul
        ones_r = persist.tile([128, 64], bf16)   # 1/Z broadcast matmul
        warm_w = persist.tile([128, 512], bf16)  # HAM warm-up fodder

        nc.vector.memset(ones_m[:], 1.0)
        nc.vector.memset(ones_r[:], 1.0)
        nc.vector.memset(warm_w[:], 0.125)
        nc.vector.memset(v_sb[:, :, :, HD:HD + 1], 1.0)  # aug ones column
        nc.vector.memset(zst[:], 1.0)

        # input DMAs on two queues: sync carries most of xT; gpsimd the
        # weights plus the tail of xT (both queues finish xT together)
        nc.sync.dma_start(bq_sb[:], bq_d.ap())
        nc.sync.dma_start(bk_sb[:], bk_d.ap())
        nc.sync.dma_start(bv_sb[:], bv_d.ap())
        for c in range(6):
            nc.sync.dma_start(xT_sb[:, c, :], xT_v[c])
        for c in range(NCH):
            nc.gpsimd.dma_start(wq_sb[:, c, :], wq_v[c])
            nc.gpsimd.dma_start(wk_sb[:, c, :], wk_v[c])
        for c in range(6, NCH):
            nc.gpsimd.dma_start(xT_sb[:, c, :], xT_v[c])
        for c in range(NCH):
            nc.gpsimd.dma_start(wv_sb[:, c, :], wv_v[c])
        for m in range(4):
            nc.gpsimd.dma_start(mask_sb[:, m, :], mask_v[m])
        for c in range(4):
            nc.gpsimd.dma_start(wp_sb[:, c, :], wp_v[c])

        # PE warm-up during the input DMA window (junk matmuls)
        for _ in range(12):
            ps = pp.tile([128, 512], f32, tag="pp")
            nc.tensor.matmul(ps[:], warm_w[:, 0:128], warm_w[:],
                             start=True, stop=True)

        # one-time broadcast of the v bias across partitions (replaces a
        # rank-1 bias matmul per v t-tile)
        bvb = persist.tile([128, 512], bf16)
        psb = pp.tile([128, 512], f32, tag="pp")
        nc.tensor.matmul(psb[:], ones_m[:], bv_sb[:], start=True, stop=True)
        nc.vector.tensor_copy(bvb[:], psb[:])

        # ---------------- emission helpers ----------------
        def qk_unit(hp, tn, t4):
            w_sb, b_sb, dst = (wq_sb, bq_sb, qT_sb) if tn == 0 else \
                              (wk_sb, bk_sb, kT_sb)
            ps = pp.tile([128, 512], f32, tag="pp")
            for c in range(NCH):
                nc.tensor.matmul(
                    ps[:], w_sb[:, c, 128 * hp:128 * hp + 128],
                    xT_sb[:, c, 512 * t4:512 * t4 + 512],
                    start=(c == 0), stop=(c == NCH - 1))
            nc.vector.tensor_scalar_add(
                dst[:, hp, 512 * t4:512 * t4 + 512], ps[:], b_sb[:, hp:hp + 1])

        def v_unit(r):
            ps = pp.tile([128, 512], f32, tag="pp")
            for c in range(NCH):
                nc.tensor.matmul(
                    ps[:], xT_sb[:, c, 128 * r:128 * r + 128], wv_sb[:, c, :],
                    start=(c == 0), stop=(c == NCH - 1))
            nc.vector.tensor_add(
                v_sb[:, r, :, 0:HD],
                ps[:].rearrange("p (h e) -> p h e", e=HD),
                bvb[:].rearrange("p (h e) -> p h e", e=HD))

        def norm_unit(hp, J):
            # broadcast both heads' Z of block J into one PSUM bank
            # (row 32J stationary, col positions 0/64), then one reciprocal
            # and one multiply normalize the whole [128, 512] yT slice
            b = 32 * J
            qs = slice(512 * J, 512 * J + 512)
            bp2 = pp.tile([128, 512], f32, tag="pp")
            nc.tensor.matmul(bp2[0:64, :], ones_r[b:b + 1, :],
                             zst[b:b + 1, 2 * hp, :], tile_position=(b, 0))
            nc.tensor.matmul(bp2[64:128, :], ones_r[b:b + 1, :],
                             zst[b:b + 1, 2 * hp + 1, :],
                             tile_position=(b, 64))
            nc.vector.reciprocal_approx_fast(bp2[:], bp2[:])
            nc.vector.tensor_mul(yT_sb[:, hp, qs], yT_sb[:, hp, qs], bp2[:])

        def p_unit(qt, co):
            ps = pp.tile([128, 512], f32, tag="pp")
            for c2 in range(4):
                nc.tensor.matmul(
                    ps[:], yT_sb[:, c2, 128 * qt:128 * qt + 128],
                    wp_sb[:, c2, 512 * co:512 * co + 512],
                    start=(c2 == 0), stop=(c2 == 3))
            ot = outp.tile([128, 512], f32, tag="ot")
            if co == 0:
                nc.vector.tensor_copy(ot[:], ps[:])
            else:
                nc.scalar.activation(ot[:], ps[:], AF.Copy)
            eng = nc.sync if (qt + co) % 2 == 0 else nc.gpsimd
            eng.dma_start(
                out_d.ap()[128 * qt:128 * qt + 128,
                           512 * co:512 * co + 512], ot[:])

        def attention_pair(hp, tasks, jhook=None):
            steps = [(J, kt) for J in range(4) for kt in range(4 * (J + 1))]
            n = len(steps)
            state = {"emitted": 0, "cur": None, "pend": None}

            def drain_to(k):
                while state["emitted"] < min(k, len(tasks)):
                    tasks[state["emitted"]]()
                    state["emitted"] += 1

            def flush():
                J, kt, p2, i0 = state["pend"]
                state["pend"] = None
                E = 4 * (J + 1)
                if kt == 0:
                    oA = opool.tile([HD + 1, 512], f32, tag="oA", name="oA")
                    oB = opool.tile([HD + 1, 512], f32, tag="oB", name="oB")
                    state["cur"] = (oA, oB)
                oA, oB = state["cur"]
                last = (kt == E - 1)
                nc.tensor.matmul(oA[:, i0:512], v_sb[:, kt, 2 * hp, :],
                                 p2[:, i0:512], start=(kt == 0), stop=last)
                nc.tensor.matmul(oB[:, i0:512], v_sb[:, kt, 2 * hp + 1, :],
                                 p2[:, 512 + i0:1024],
                                 start=(kt == 0), stop=last)
                if last:
                    qs = slice(512 * J, 512 * J + 512)
                    nc.scalar.activation(yT_sb[0:64, hp, qs], oA[0:HD, :],
                                         AF.Copy)
                    nc.vector.tensor_copy(yT_sb[64:128, hp, qs], oB[0:HD, :])
                    b = 32 * J
                    nc.vector.tensor_copy(zst[b:b + 1, 2 * hp, :],
                                          oA[HD:HD + 1, :])
                    nc.vector.tensor_copy(zst[b:b + 1, 2 * hp + 1, :],
                                          oB[HD:HD + 1, :])
                    if jhook is not None:
                        jhook(J, drain_to)

            for idx, (J, kt) in enumerate(steps):
                ks = slice(128 * kt, 128 * kt + 128)
                i0 = 128 * (kt - 4 * J) if kt >= 4 * J else 0
                s2 = spool.tile([128, 1024], f32, tag="s2")
                nc.tensor.matmul(s2[:, i0:512], kT_sb[0:64, hp, ks],
                                 qT_sb[0:64, hp, 512 * J + i0:512 * J + 512],
                                 tile_position=(0, 0))
                nc.tensor.matmul(s2[:, 512 + i0:1024], kT_sb[64:128, hp, ks],
                                 qT_sb[64:128, hp,
                                       512 * J + i0:512 * J + 512],
                                 tile_position=(64, 0))
                p2 = ppool.tile([128, 1024], bf16, tag="p2")
                s2v = s2[:].rearrange("p (h q) -> p h q", q=512)
                p2v = p2[:].rearrange("p (h q) -> p h q", q=512)
                if kt % 3 == 1:   # Schraudolph exp on the vector engine
                    nc.vector.tensor_scalar(
                        p2v[:, :, i0:512].bitcast(i16), s2v[:, :, i0:512],
                        EXP_A, EXP_B, ALU.mult, ALU.add)
                else:             # spline exp on the scalar engine
                    nc.scalar.activation(p2v[:, :, i0:512], s2v[:, :, i0:512],
                                         AF.Exp, scale=SCALE)
                if kt >= 4 * J:  # diagonal block: causal mask
                    # only columns [i0, i0+128) straddle the triangle; the
                    # rest of the tile is all-ones
                    m = kt - 4 * J
                    ie = min(i0 + 128, 512)
                    nc.vector.tensor_mul(p2[:, i0:ie], p2[:, i0:ie],
                                         mask_sb[:, m, i0:ie])
                    nc.vector.tensor_mul(p2[:, 512 + i0:512 + ie],
                                         p2[:, 512 + i0:512 + ie],
                                         mask_sb[:, m, i0:ie])
                drain_to(len(tasks) * (idx + 1) // n)
                if state["pend"] is not None:
                    flush()
                state["pend"] = (J, kt, p2, i0)
            flush()
            drain_to(len(tasks))
            return drain_to

        # ---------------- schedule ----------------
        # prologue: projections for pair 0, first half of v
        for tn in range(2):
            for t4 in range(4):
                qk_unit(0, tn, t4)
        for r in range(8):
            v_unit(r)

        def mk_v(r):
            return lambda: v_unit(r)

        def mk_qk(hp, tn, t4):
            return lambda: qk_unit(hp, tn, t4)

        def mk_norm(hp, J):
            return lambda: norm_unit(hp, J)

        # pair 0: rest of v + projections for pair 1
        attention_pair(0, [mk_v(r) for r in range(8, 16)] +
                       [mk_qk(1, tn, t4) for tn in range(2)
                        for t4 in range(4)])
        # pair 1: projections for pair 2, then pair-0 normalization
        attention_pair(1, [mk_qk(2, tn, t4) for tn in range(2)
                           for t4 in range(4)] +
                       [mk_norm(0, J) for J in range(4)])
        # pair 2: projections for pair 3, then pair-1 normalization
        attention_pair(2, [mk_qk(3, tn, t4) for tn in range(2)
                           for t4 in range(4)] +
                       [mk_norm(1, J) for J in range(4)])

        # pair 3: pair-2 normalization tasks (J-major so the jhook can
        # force-drain them per J), plus per-J appended work: pair-3
        # normalization and the output projection of that J's token rows
        p3_tasks = [mk_norm(2, J) for J in range(4)]

        def mk_p(qt, co):
            return lambda: p_unit(qt, co)

        def jhook3(J, drain_to):
            drain_to(J + 1)              # norm(pair 2, J) done
            p3_tasks.append(mk_norm(3, J))
            for qt in range(4 * J, 4 * J + 4):
                for co in range(2):
                    p3_tasks.append(mk_p(qt, co))

        attention_pair(3, p3_tasks, jhook=jhook3)

    nc.compile()
    return nc


def prep_in_maps(x, Wq, bq, Wk, bk, Wv, bv, Wp, bp):
    x = np.asarray(x, dtype=np.float32)
    Wq = np.asarray(Wq, dtype=np.float32)
    Wk = np.asarray(Wk, dtype=np.float32)
    Wv = np.asarray(Wv, dtype=np.float32)
    Wp = np.asarray(Wp, dtype=np.float32)
    bq = np.asarray(bq, dtype=np.float32)
    bk = np.asarray(bk, dtype=np.float32)
    bv = np.asarray(bv, dtype=np.float32)

    bf = ml_dtypes.bfloat16
    WqT, WkT, WvT, WpT = Wq.T, Wk.T, Wv.T, Wp.T

    kk = np.arange(128)[:, None]
    qq = np.arange(512)[None, :]
    mask = np.ascontiguousarray(np.concatenate(
        [(128 * m + kk <= qq) for m in range(4)], axis=0).astype(bf))

    xTs = [np.ascontiguousarray(x[b].T).astype(bf) for b in range(B)]
    wq_s, wk_s, wv_s, wp_s, bq_s, bk_s, bv_s = [], [], [], [], [], [], []
    for hg in range(2):
        sl = slice(512 * hg, 512 * hg + 512)
        wq_s.append(np.ascontiguousarray(WqT[:, sl]).astype(bf))
        wk_s.append(np.ascontiguousarray(WkT[:, sl]).astype(bf))
        wv_s.append(np.ascontiguousarray(WvT[:, sl]).astype(bf))
        wp_s.append(np.ascontiguousarray(WpT[sl, :]).astype(bf))
        bq_s.append(np.ascontiguousarray(bq[sl].reshape(4, 128).T))
        bk_s.append(np.ascontiguousarray(bk[sl].reshape(4, 128).T))
        bv_s.append(np.ascontiguousarray(bv[sl].reshape(1, 512)).astype(bf))

    in_maps = []
    for core in range(NCORES):
        b, hg = core // 2, core % 2
        in_maps.append({
            "xT": xTs[b],
            "wqT": wq_s[hg], "wkT": wk_s[hg], "wvT": wv_s[hg],
            "wpT": wp_s[hg],
            "bq2": bq_s[hg], "bk2": bk_s[hg], "bv2": bv_s[hg],
            "mask": mask,
        })
    return in_maps


def kernel(x, Wq, bq, Wk, bk, Wv, bv, Wp, bp, **_ignored):
    global last_result
    bp = np.asarray(bp, dtype=np.float32)
    in_maps = prep_in_maps(x, Wq, bq, Wk, bk, Wv, bv, Wp, bp)

    if "nc" not in _compiled:
        _compiled["nc"] = _build()
    nc = _compiled["nc"]

    last_result = bass_utils.run_bass_kernel_spmd(
        nc, in_maps, core_ids=list(range(NCORES)))

    out = np.empty((B, T, C), dtype=np.float32)
    for b in range(B):
        out[b] = last_result.results[2 * b]["out"]
        out[b] += last_result.results[2 * b + 1]["out"]
    out += bp[None, None, :]
    return out
